# revision 17
# baseline (speedup 1.0000x reference)
"""ExpertPreferredRouter on 8 TRN2 NeuronCores — folded-128 log-space version.

Structure (per core; batch b = core%4, half h = core//4, 2048 tokens/core):
  - Fold: local token lt -> partition group u = lt//1024, col = lt%1024.
    All wave tensors are [128, *] (expert j + 64u on partitions) so DVE and PE
    run at full partition width.  The algorithm is value-based and therefore
    column/partition-permutation invariant; each core's view of the full 4096
    tokens is [own fold | partner fold] in AllGather replica order.
  - Phase A: logits into one [128, 1024] PSUM via dual weight blocks
    ([wt|0] for u=0 tokens, [0|wt] for u=1); log-softmax s = logits - ln(Z)
    (Ln on the scalar engine; no single-partition DVE reciprocal).
  - Init: 8 max8/match_replace rounds -> per-(j,u) top-64; AllGather of the
    s-tile overlapped under init; second small AllGather of candidates;
    merged [64,256] pool -> exact t0 = global 64th per expert.
  - Waves: steal mask via block-diag strict-upper matmul on [128, *] tiles,
    fused compare+count, per-chunk max8 candidates merged to a [64,32] pool,
    threshold descends up to DMAX=16 ranks/wave (cand17 = [t, top16(pool)]).
  - Final masked pass -> M (priority matmul) and P = exp(selected s).
"""
import os
import sys
import types

import numpy as np

B, N, D, E = 4, 4096, 4096, 64
H = N // 2            # tokens per core (half a batch)
NF = H                # folded full-row width: [128, NF] covers all N tokens
NCORES = 8
WAVES = 11            # content waves; sim (dmax=16 pool) exact at 11+final
DMAX = 16
BIGSEL = float(2.0 ** 100)

TRACE = False         # set True (e.g. by test.py) to capture NTFF timing
LAST_EXEC_NS = None

_cache = {}

# token unfold maps (host side): token n -> (partition group u, column)
_n = np.arange(N)
_u_of_n = (_n // 1024) % 2
_col_of_n = _n % 1024 + 1024 * (_n // 2048)


def _install_ntff_hook():
    if "antenv.axon_hooks" in sys.modules:
        return
    mod = types.ModuleType("antenv.axon_hooks")
    state = {"hook": None}
    mod.set_axon_ntff_profile_hook = lambda h: state.__setitem__("hook", h)
    mod.get_axon_ntff_profile_hook = lambda: state["hook"]
    sys.modules["antenv.axon_hooks"] = mod
    try:
        import antenv
        antenv.axon_hooks = mod
    except ImportError:
        pass
    try:
        from trn_agent_boot.trn_boot import _ntff_profile_via_ctypes
        mod.set_axon_ntff_profile_hook(
            _ntff_profile_via_ctypes("/opt/axon/libaxon_pjrt.so")
        )
    except Exception:
        pass


def _build_program():
    import concourse.bacc as bacc
    import concourse.mybir as mybir
    from concourse.tile import TileContext
    from concourse.masks import make_identity

    f32 = mybir.dt.float32
    bf16 = mybir.dt.bfloat16
    i32 = mybir.dt.int32
    Alu = mybir.AluOpType
    Act = mybir.ActivationFunctionType

    nc = bacc.Bacc("TRN2", target_bir_lowering=False, num_devices=NCORES)

    xt = nc.dram_tensor("xt", [D, H], f32, kind="ExternalInput")
    wt = nc.dram_tensor("wt", [D, E], f32, kind="ExternalInput")
    mo = nc.dram_tensor("mo", [2, NF], f32, kind="ExternalOutput")
    po = nc.dram_tensor("po", [2, NF], f32, kind="ExternalOutput")
    co = nc.dram_tensor("co", [E, 1], f32, kind="ExternalOutput")
    DEBUG = bool(int(os.environ.get("KDEBUG", "0")))
    if DEBUG:
        ro = nc.dram_tensor("ro", [128, NF], f32, kind="ExternalOutput")
        t0o = nc.dram_tensor("t0o", [E, 1], f32, kind="ExternalOutput")

    with TileContext(nc) as tc:
        with (
            tc.tile_pool(name="persist", bufs=1) as pp,
            tc.tile_pool(name="work", bufs=1) as wp,
            tc.tile_pool(name="stream", bufs=4) as sp,
            tc.tile_pool(name="small", bufs=2) as smp,
            tc.tile_pool(name="dram", bufs=1, space="DRAM") as dp,
        ):
            # ---------------- constants (issued early; overlap phase A DMA) ----
            pi = pp.tile([128, 1], i32, tag="pi")
            nc.gpsimd.iota(pi[:], pattern=[[0, 1]], base=0, channel_multiplier=1)
            pif = pp.tile([128, 1], f32, tag="pif")
            nc.vector.tensor_copy(pif[:], pi[:])
            hp = pp.tile([128, 1], f32, tag="hp")
            nc.vector.tensor_scalar(hp[:], pif[:], 64.0, None, op0=Alu.is_ge)
            jp = pp.tile([128, 1], f32, tag="jp")
            nc.vector.scalar_tensor_tensor(
                jp[:], hp[:], -64.0, pif[:], op0=Alu.mult, op1=Alu.add
            )
            ci = pp.tile([128, 128], i32, tag="ci")
            nc.gpsimd.iota(ci[:], pattern=[[1, 128]], base=0, channel_multiplier=0)
            cif = pp.tile([128, 128], f32, tag="cif")
            nc.vector.tensor_copy(cif[:], ci[:])
            hc = pp.tile([128, 128], f32, tag="hc")
            nc.vector.tensor_scalar(hc[:], cif[:], 64.0, None, op0=Alu.is_ge)
            jcol = pp.tile([128, 128], f32, tag="jcol")
            nc.vector.scalar_tensor_tensor(
                jcol[:], hc[:], -64.0, cif[:], op0=Alu.mult, op1=Alu.add
            )
            cond1 = wp.tile([128, 128], f32, tag="cond1")
            nc.vector.tensor_scalar(cond1[:], jcol[:], jp[:], None, op0=Alu.is_lt)
            cond2 = wp.tile([128, 128], f32, tag="cond2")
            nc.vector.tensor_scalar(cond2[:], hc[:], hp[:], None, op0=Alu.is_equal)
            # ustrict[p=j'+64h', c=j+64h] = -BIG if j' > j and h' == h
            ustrict = pp.tile([128, 128], bf16, tag="ustrict")
            nc.vector.scalar_tensor_tensor(
                ustrict[:], cond1[:], -BIGSEL, cond2[:], op0=Alu.mult, op1=Alu.mult
            )
            ident128 = pp.tile([128, 128], f32, tag="ident128")
            make_identity(nc, ident128)
            # Wz [128, 2]: col0 = 1-hp, col1 = hp (u-half fold for column sums)
            Wz = pp.tile([128, 2], f32, tag="Wz")
            nc.vector.tensor_scalar(
                Wz[:, 0:1], hp[:], -1.0, 1.0, op0=Alu.mult, op1=Alu.add
            )
            nc.vector.tensor_copy(Wz[:, 1:2], hp[:])
            # Wb [2, 128]: row p -> 1 where column's u-half == p (broadcast up)
            ci2 = pp.tile([2, 128], i32, tag="ci2")
            nc.gpsimd.iota(ci2[:], pattern=[[1, 128]], base=0, channel_multiplier=0)
            ci2f = pp.tile([2, 128], f32, tag="ci2f")
            nc.vector.tensor_copy(ci2f[:], ci2[:])
            hc2 = pp.tile([2, 128], f32, tag="hc2")
            nc.vector.tensor_scalar(hc2[:], ci2f[:], 64.0, None, op0=Alu.is_ge)
            pi2 = pp.tile([2, 1], i32, tag="pi2")
            nc.gpsimd.iota(pi2[:], pattern=[[0, 1]], base=0, channel_multiplier=1)
            pi2f = pp.tile([2, 1], f32, tag="pi2f")
            nc.vector.tensor_copy(pi2f[:], pi2[:])
            Wb = pp.tile([2, 128], f32, tag="Wb")
            nc.vector.tensor_scalar(Wb[:], hc2[:], pi2f[:], None, op0=Alu.is_equal)
            # Wm [128, 2] bf16: col u = jp * (hp == u) (priority readout)
            Wm = pp.tile([128, 2], bf16, tag="Wm")
            nc.vector.tensor_tensor(Wm[:, 0:1], jp[:], Wz[:, 0:1], op=Alu.mult)
            nc.vector.tensor_tensor(Wm[:, 1:2], jp[:], hp[:], op=Alu.mult)
            # iota17 [64, 17] for cand indexing
            i17 = pp.tile([64, 17], i32, tag="i17")
            nc.gpsimd.iota(i17[:], pattern=[[1, 17]], base=0, channel_multiplier=0)
            iota17 = pp.tile([64, 17], f32, tag="iota17")
            nc.vector.tensor_copy(iota17[:], i17[:])

            # ---------------- Phase A: logits -> log-softmax ----------------
            # wtbig[p, dc, 0:64] = wt[dc*128+p, :]; [64:128] = 0; [128:192] = wt
            # WtLow(dc) = wtbig[:, dc, 0:128]  -> expert channels 0-63 (u=0)
            # WtHigh(dc) = wtbig[:, dc, 64:192] -> channels 64-127 (u=1)
            r2 = pp.tile([128, NF], f32, tag="r2")   # full folded s
            with tc.tile_pool(name="phA", bufs=1) as pa:
                # wt_sb[p, dc*64+e] = wt[dc*128+p, e]; u=1 logits land on psum
                # partitions 64:128 via output-AP partition offset (64-out fp32
                # matmuls run 2x faster than 128-out: 429ns vs 852ns per 512)
                wt_sb = pa.tile([128, 32 * E], f32, tag="wt")
                nc.sync.dma_start(
                    wt_sb[:].rearrange("p (c e) -> p c e", e=E),
                    wt[:].rearrange("(c p) e -> p c e", p=128),
                )
                with tc.tile_pool(name="plog", bufs=1, space="PSUM") as plog_pool:
                    psumA = plog_pool.tile([128, 1024], f32, tag="plog")
                    for dd in range(16):
                        # 2MB double-chunk per DMA; alternate trigger engines so
                        # descriptor issue does not serialize on one sequencer
                        xchunk = sp.tile([128, 2 * H], f32, tag="xchunk")
                        eng = nc.sync if dd % 2 == 0 else nc.scalar
                        eng.dma_start(
                            xchunk[:].rearrange("p (s t) -> p s t", t=H),
                            xt[dd * 256: (dd + 1) * 256, :].rearrange(
                                "(s p) t -> p s t", p=128
                            ),
                        )
                        for sub in range(2):
                            dc = dd * 2 + sub
                            wsl = slice(dc * E, (dc + 1) * E)
                            xbase = sub * H
                            for cg in range(2):
                                sl = slice(xbase + cg * 512, xbase + (cg + 1) * 512)
                                nc.tensor.matmul(
                                    psumA[0:64, cg * 512: (cg + 1) * 512],
                                    wt_sb[:, wsl],
                                    xchunk[:, sl],
                                    start=(dc == 0), stop=(dc == 31),
                                )
                                nc.tensor.matmul(
                                    psumA[64:128, cg * 512: (cg + 1) * 512],
                                    wt_sb[:, wsl],
                                    xchunk[:, xbase + 1024 + cg * 512: xbase + 1024 + (cg + 1) * 512],
                                    start=(dc == 0), stop=(dc == 31),
                                )
                    # Z per (u, col) by folding expert partitions; then ln
                    expT = wp.tile([128, 1024], f32, tag="expT")
                    nc.scalar.activation(expT[:], psumA[:], Act.Exp)
                    lnZ = wp.tile([2, 1024], f32, tag="lnZ")
                    with tc.tile_pool(name="pz", bufs=1, space="PSUM") as pz_pool:
                        pz = pz_pool.tile([2, 1024], f32, tag="pz")
                        for cg in range(2):
                            sl = slice(cg * 512, (cg + 1) * 512)
                            nc.tensor.matmul(
                                pz[:, sl], Wz[:], expT[:, sl], start=True, stop=True
                            )
                        nc.scalar.activation(lnZ[:], pz[:], Act.Ln)
                    with tc.tile_pool(name="pb", bufs=1, space="PSUM") as pb_pool:
                        lnZb = pb_pool.tile([128, 1024], f32, tag="lnZb")
                        for cg in range(2):
                            sl = slice(cg * 512, (cg + 1) * 512)
                            nc.tensor.matmul(
                                lnZb[:, sl], Wb[:], lnZ[:, sl], start=True, stop=True
                            )
                        # DVE can read only one PSUM input; stage lnZb via the
                        # (otherwise idle) scalar engine
                        lnZbs = wp.tile([128, 1024], f32, tag="lnZbs")
                        nc.scalar.activation(lnZbs[:], lnZb[:], Act.Copy)
                        # s (own fold) = logits - lnZ -> r2 cols 0:1024
                        nc.vector.tensor_tensor(
                            r2[:, 0:1024], psumA[:], lnZbs[:], op=Alu.subtract
                        )

            # ---------------- AllGather 1: s-tile (overlaps with init) -------
            agin1 = dp.tile([128, 1024], f32)
            agout1 = dp.tile([2, 128, 1024], f32)
            nc.scalar.dma_start(agin1[:], r2[:, 0:1024])
            nc.gpsimd.collective_compute(
                "AllGather",
                mybir.AluOpType.bypass,
                replica_groups=[[0, 4], [1, 5], [2, 6], [3, 7]],
                ins=[agin1.opt()],
                outs=[agout1.opt()],
            )

            # ---------------- Init: per-(j,u) top-64 of own half -------------
            cand2 = wp.tile([128, 64], f32, tag="cand2")
            wrkA = wp.tile([128, 1024], f32, tag="wrkA")
            wrkB = wp.tile([128, 1024], f32, tag="wrkB")
            nc.vector.tensor_copy(wrkA[:], r2[:, 0:1024])
            cur, nxt = wrkA, wrkB
            for rnd in range(8):
                m8 = smp.tile([128, 8], f32, tag="m8")
                nc.vector.max(m8[:], cur[:])
                nc.vector.tensor_copy(cand2[:, rnd * 8: rnd * 8 + 8], m8[:])
                if rnd < 7:
                    nc.vector.match_replace(
                        out=nxt[:], in_to_replace=m8[:], in_values=cur[:],
                        imm_value=-1e38,
                    )
                    cur, nxt = nxt, cur

            # gathered s-tiles -> r2 full (replica order; own rewrite harmless)
            for g in range(2):
                nc.sync.dma_start(
                    r2[:, g * 1024: (g + 1) * 1024], agout1[g, :, :]
                )

            # fold own 2 sorted-64 lists -> [64, 128] via partition-offset
            # copies; merge-sort to a single sorted top-64 (runs under
            # AllGather-1's link transfer)
            poolsb = wp.tile([64, 128], f32, tag="poolsb")
            nc.vector.tensor_copy(poolsb[:, 0:64], cand2[0:64, :])
            nc.vector.tensor_copy(poolsb[:, 64:128], cand2[64:128, :])
            poolwk = wp.tile([64, 128], f32, tag="poolwk")
            candOwn = wp.tile([64, 64], f32, tag="candOwn")
            curp, nxtp = poolsb, poolwk
            for rnd in range(8):
                m8b = smp.tile([64, 8], f32, tag="m8b")
                nc.vector.max(m8b[:], curp[:])
                nc.vector.tensor_copy(candOwn[:, rnd * 8: rnd * 8 + 8], m8b[:])
                if rnd < 7:
                    nc.vector.match_replace(
                        out=nxtp[:], in_to_replace=m8b[:], in_values=curp[:],
                        imm_value=-1e38,
                    )
                    curp, nxtp = nxtp, curp

            # ---------------- AllGather 2: merged own top-64 (16KB) ----------
            agin2 = dp.tile([64, 64], f32)
            agout2 = dp.tile([2, 64, 64], f32)
            nc.scalar.dma_start(agin2[:], candOwn[:])
            nc.gpsimd.collective_compute(
                "AllGather",
                mybir.AluOpType.bypass,
                replica_groups=[[0, 4], [1, 5], [2, 6], [3, 7]],
                ins=[agin2.opt()],
                outs=[agout2.opt()],
            )
            candAB = wp.tile([64, 128], f32, tag="candAB")
            for g in range(2):
                nc.scalar.dma_start(
                    candAB[:, g * 64: (g + 1) * 64], agout2[g, :, :]
                )

            # t0 = 64th of union of two sorted-64 lists:
            # max_{i+j=64} min(A_i, B_j) with A_0 = B_0 = +inf
            apad = wp.tile([64, 65], f32, tag="apad")
            brev = wp.tile([64, 65], f32, tag="brev")
            nc.vector.memset(apad[:, 0:1], 1e38)
            nc.vector.tensor_copy(apad[:, 1:], candAB[:, 0:64])
            nc.vector.memset(brev[:, 64:], 1e38)
            nc.vector.tensor_copy(brev[:, 0:64], candAB[:, 64:128][:, ::-1])
            tmin = wp.tile([64, 65], f32, tag="tmin")
            nc.vector.tensor_tensor(tmin[:], apad[:], brev[:], op=Alu.min)
            t_vec = pp.tile([64, 1], f32, tag="t")
            nc.vector.tensor_reduce(
                t_vec[:], tmin[:], axis=mybir.AxisListType.X, op=Alu.max
            )

            if DEBUG:
                nc.sync.dma_start(ro[:], r2[:])
                nc.sync.dma_start(t0o[:], t_vec[:])

            # t broadcast to both u-halves (partition-offset copies)
            t2sb = pp.tile([128, 1], f32, tag="t2sbi", name="t2_init")
            nc.vector.tensor_copy(t2sb[0:64, :], t_vec[:])
            nc.vector.tensor_copy(t2sb[64:128, :], t_vec[:])

            # wave 0: raw claims mask only (steal matmul would be on zeros)
            msk = pp.tile([128, NF], bf16, tag="mskA", name="msk_init")
            nc.vector.tensor_scalar(msk[:], r2[:], t2sb[:], None, op0=Alu.is_ge)

            candp = smp.tile([128, 16], f32, tag="candp")
            NCH = 2
            CW = NF // NCH  # 1024

            # ---------------- waves ----------------
            # pm PSUM pool is global (bufs=3 = 6 banks): chunk-0's identity
            # matmul for the NEXT wave is prefetched while the current wave's
            # DVE work runs, so after mskn lands only the (bf16, fast) steal
            # matmul remains on the critical path.
            wave_pools = tc.tile_pool(name="pmglob", bufs=2, space="PSUM")
            cw_pools = tc.tile_pool(name="pcw", bufs=1, space="PSUM")
            pmp = wave_pools.__enter__()
            pcw = cw_pools.__enter__()
            pm_pre = pmp.tile([128, CW], f32, tag="pm", name="pm_w1_c0")
            for cg in range(CW // 512):
                psl = slice(cg * 512, (cg + 1) * 512)
                nc.tensor.matmul(
                    pm_pre[:, psl], ident128[:], r2[:, psl],
                    start=True, stop=False,
                )
            for wv in range(1, WAVES + 2):
                last = wv == WAVES + 1
                cntp2 = smp.tile([128, NCH], f32, tag="cntp2")
                if last:
                    msk01 = pp.tile([128, NF], bf16, tag="msk01")
                for ch in range(NCH):
                    if ch == 0:
                        pm = pm_pre
                    else:
                        pm = pmp.tile([128, CW], f32, tag="pm",
                                      name=f"pm_w{wv}_c{ch}")
                    for cg in range(CW // 512):
                        sl = slice(ch * CW + cg * 512, ch * CW + (cg + 1) * 512)
                        psl = slice(cg * 512, (cg + 1) * 512)
                        if ch != 0:
                            nc.tensor.matmul(
                                pm[:, psl], ident128[:], r2[:, sl],
                                start=True, stop=False,
                            )
                        nc.tensor.matmul(
                            pm[:, psl], ustrict[:], msk[:, sl],
                            start=False, stop=True,
                        )
                    sl = slice(ch * CW, (ch + 1) * CW)
                    if last:
                        nc.vector.tensor_scalar(
                            msk01[:, sl], pm[:], t2sb[:], None,
                            op0=Alu.is_ge, op1=Alu.add,
                            accum_out=cntp2[:, ch: ch + 1],
                        )
                    else:
                        # selB = BIG * (pm >= t); counts come back scaled by
                        # BIG (undone in the chain).  The knockout subtract
                        # runs on the Pool engine (SBUF-only), fed by a scalar-
                        # engine PSUM->SBUF stage, so DVE drops the stt pass.
                        selB = smp.tile([128, CW], f32, tag="selB")
                        nc.vector.tensor_scalar(
                            selB[:], pm[:], t2sb[:], BIGSEL,
                            op0=Alu.is_ge, op1=Alu.mult,
                            accum_out=cntp2[:, ch: ch + 1],
                        )
                        pmsb = smp.tile([128, CW], f32, tag="pmsb")
                        nc.scalar.activation(pmsb[:], pm[:], Act.Copy)
                        wchunk = smp.tile([128, CW], f32, tag="wchunk")
                        nc.gpsimd.tensor_tensor(
                            wchunk[:], pmsb[:], selB[:], op=Alu.subtract
                        )
                        nc.vector.max(candp[:, ch * 8: ch * 8 + 8], wchunk[:])

                # prefetch next wave chunk-0 identity sums (no data deps; runs
                # on PE while DVE processes this wave)
                if not last:
                    pm_pre = pmp.tile([128, CW], f32, tag="pm",
                                      name=f"pm_w{wv + 1}_c0")
                    for cg in range(CW // 512):
                        psl = slice(cg * 512, (cg + 1) * 512)
                        nc.tensor.matmul(
                            pm_pre[:, psl], ident128[:], r2[:, psl],
                            start=True, stop=False,
                        )

                # counts: reduce chunks, then fold the two u-halves with
                # partition-offset copy + same-base add (no PE round trip)
                cnt2 = smp.tile([128, 1], f32, tag="cnt2")
                nc.vector.tensor_reduce(
                    cnt2[:], cntp2[:], axis=mybir.AxisListType.X, op=Alu.add
                )
                if True:
                    cnthi = smp.tile([64, 1], f32, tag="cnthi")
                    nc.vector.tensor_copy(cnthi[:], cnt2[64:128, :])
                    cntf = smp.tile([64, 1], f32, tag="cntfsb")
                    nc.vector.tensor_tensor(
                        cntf[:], cnt2[0:64, :], cnthi[:], op=Alu.add
                    )
                    if last:
                        nc.sync.dma_start(co[:], cntf[:])
                        msk = msk01
                        break

                    # candidate pool [64, 32] -> top-16 into cand17[1:17]
                    cand17 = smp.tile([64, 17], f32, tag="cand17")
                    nc.vector.tensor_copy(cand17[:, 0:1], t_vec[:])
                    pool32 = smp.tile([64, 32], f32, tag="pool32")
                    nc.vector.tensor_copy(pool32[:, 0:16], candp[0:64, :])
                    nc.vector.tensor_copy(pool32[:, 16:32], candp[64:128, :])
                    nc.vector.max(cand17[:, 1:9], pool32[:])
                    pool32b = smp.tile([64, 32], f32, tag="pool32b")
                    nc.vector.match_replace(
                        out=pool32b[:], in_to_replace=cand17[:, 1:9],
                        in_values=pool32[:], imm_value=-1e38,
                    )
                    nc.vector.max(cand17[:, 9:17], pool32b[:])

                    # d = clamp(64 - cnt, 0, DMAX); t = cand17[d]
                    dmv = smp.tile([64, 1], f32, tag="dmv")
                    nc.vector.tensor_scalar(
                        dmv[:], cntf[:], -(2.0 ** -100), 64.0,
                        op0=Alu.mult, op1=Alu.add,
                    )
                    nc.vector.tensor_scalar_min(dmv[:], dmv[:], float(DMAX))
                    nc.vector.tensor_scalar_max(dmv[:], dmv[:], 0.0)
                    oh = smp.tile([64, 17], f32, tag="oh")
                    nc.vector.tensor_scalar(
                        oh[:], iota17[:], dmv[:], None, op0=Alu.is_equal
                    )
                    tsel = smp.tile([64, 17], f32, tag="tsel")
                    nc.vector.tensor_tensor(tsel[:], oh[:], cand17[:], op=Alu.mult)
                    t_vec = pp.tile([64, 1], f32, tag=f"t{wv % 2}", name=f"tvec{wv}")
                    nc.vector.tensor_reduce(
                        t_vec[:], tsel[:], axis=mybir.AxisListType.X, op=Alu.add
                    )
                    t2sb = pp.tile([128, 1], f32, tag=f"t2sb{wv % 2}",
                                   name=f"t2_{wv}")
                    nc.vector.tensor_copy(t2sb[0:64, :], t_vec[:])
                    nc.vector.tensor_copy(t2sb[64:128, :], t_vec[:])
                mskn = pp.tile([128, NF], bf16, tag=f"msk{wv % 2}", name=f"mskn{wv}")
                for ch in range(NCH):
                    sl = slice(ch * CW, (ch + 1) * CW)
                    nc.vector.tensor_scalar(
                        mskn[:, sl], r2[:, sl], t2sb[:], None, op0=Alu.is_ge
                    )
                msk = mskn

            cw_pools.__exit__(None, None, None)
            wave_pools.__exit__(None, None, None)

            # ---------------- outputs ----------------
            psel = wp.tile([128, NF], f32, tag="psel")
            nc.vector.tensor_tensor(psel[:], r2[:], msk[:], op=Alu.mult)
            mo_sb = wp.tile([2, NF], f32, tag="mo")
            po_sb = wp.tile([2, NF], f32, tag="po")
            with tc.tile_pool(name="pout", bufs=4, space="PSUM") as pop:
                for ch in range(NF // 512):
                    sl = slice(ch * 512, (ch + 1) * 512)
                    pmm = pop.tile([2, 512], f32, tag="pmm")
                    nc.tensor.matmul(
                        pmm[:], Wm[:], msk[:, sl], start=True, stop=True
                    )
                    nc.vector.tensor_copy(mo_sb[:, sl], pmm[:])
                    ppp = pop.tile([2, 512], f32, tag="ppp")
                    nc.tensor.matmul(
                        ppp[:], Wz[:], psel[:, sl], start=True, stop=True
                    )
                    nc.scalar.activation(po_sb[:, sl], ppp[:], Act.Exp)
            nc.sync.dma_start(mo[:], mo_sb[:])
            nc.sync.dma_start(po[:], po_sb[:])

    nc.compile()
    return nc


def kernel(x, W, c):
    global LAST_EXEC_NS
    from concourse import bass_utils

    x = np.asarray(x, dtype=np.float32)
    W = np.asarray(W, dtype=np.float32)

    if "nc" not in _cache:
        _cache["nc"] = _build_program()
    nc = _cache["nc"]

    wt_host = np.ascontiguousarray(W.T)  # [D, E]
    in_maps = []
    for core in range(NCORES):
        b, h = core % B, core // B
        xt_host = np.ascontiguousarray(x[b, h * H: (h + 1) * H, :].T)  # [D, H]
        in_maps.append({"xt": xt_host, "wt": wt_host})

    trace = TRACE
    if trace:
        _install_ntff_hook()
    res = bass_utils.run_bass_kernel_spmd(
        nc, in_maps, core_ids=list(range(NCORES)), trace=trace
    )
    LAST_EXEC_NS = res.exec_time_ns

    M = np.zeros((B, N), dtype=np.int32)
    P = np.zeros((B, N), dtype=np.float32)
    for b in range(B):
        out = res.results[b]
        cnt = out["co"][:, 0]
        if not np.allclose(cnt, 64.0):
            print(f"[kernel] WARNING: batch {b} expert counts != 64: "
                  f"min={cnt.min()} max={cnt.max()}", file=sys.stderr)
        # core b has h=0: cols 0:1024 = tokens 0:2048 folded (u = lt//1024,
        # col = lt%1024); cols 1024:2048 = tokens 2048:4096 folded.
        m2 = out["mo"]  # [2, 2048]
        p2 = out["po"]
        M[b, :] = np.rint(m2[_u_of_n, _col_of_n]).astype(np.int32)
        P[b, :] = p2[_u_of_n, _col_of_n].astype(np.float32)
    return M, P


# revision 19
# speedup vs baseline: 1.0636x; 1.0636x over previous
"""ExpertPreferredRouter on 8 TRN2 NeuronCores — folded-128 log-space version.

Structure (per core; batch b = core%4, half h = core//4, 2048 tokens/core):
  - Fold: local token lt -> partition group u = lt//1024, col = lt%1024.
    All wave tensors are [128, *] (expert j + 64u on partitions) so DVE and PE
    run at full partition width.  The algorithm is value-based and therefore
    column/partition-permutation invariant; each core's view of the full 4096
    tokens is [own fold | partner fold] in AllGather replica order.
  - Phase A: logits into one [128, 1024] PSUM via dual weight blocks
    ([wt|0] for u=0 tokens, [0|wt] for u=1); log-softmax s = logits - ln(Z)
    (Ln on the scalar engine; no single-partition DVE reciprocal).
  - Init: 8 max8/match_replace rounds -> per-(j,u) top-64; AllGather of the
    s-tile overlapped under init; second small AllGather of candidates;
    merged [64,256] pool -> exact t0 = global 64th per expert.
  - Waves: steal mask via block-diag strict-upper matmul on [128, *] tiles,
    fused compare+count, per-chunk max8 candidates merged to a [64,32] pool,
    threshold descends up to DMAX=16 ranks/wave (cand17 = [t, top16(pool)]).
  - Final masked pass -> M (priority matmul) and P = exp(selected s).
"""
import os
import sys
import types

import numpy as np

B, N, D, E = 4, 4096, 4096, 64
H = N // 2            # tokens per core (half a batch)
NF = H                # folded full-row width: [128, NF] covers all N tokens
NCORES = 8
WAVES = 11            # content waves; sim (dmax=16 pool) exact at 11+final
DMAX = 16
BIGSEL = float(2.0 ** 100)

TRACE = False         # set True (e.g. by test.py) to capture NTFF timing
LAST_EXEC_NS = None

_cache = {}

# token unfold maps (host side): token n -> (partition group u, column)
_n = np.arange(N)
_u_of_n = (_n // 1024) % 2
_col_of_n = _n % 1024 + 1024 * (_n // 2048)


def _install_ntff_hook():
    if "antenv.axon_hooks" in sys.modules:
        return
    mod = types.ModuleType("antenv.axon_hooks")
    state = {"hook": None}
    mod.set_axon_ntff_profile_hook = lambda h: state.__setitem__("hook", h)
    mod.get_axon_ntff_profile_hook = lambda: state["hook"]
    sys.modules["antenv.axon_hooks"] = mod
    try:
        import antenv
        antenv.axon_hooks = mod
    except ImportError:
        pass
    try:
        from trn_agent_boot.trn_boot import _ntff_profile_via_ctypes
        mod.set_axon_ntff_profile_hook(
            _ntff_profile_via_ctypes("/opt/axon/libaxon_pjrt.so")
        )
    except Exception:
        pass


def _build_program():
    import concourse.bacc as bacc
    import concourse.mybir as mybir
    from concourse.tile import TileContext
    from concourse.masks import make_identity

    f32 = mybir.dt.float32
    bf16 = mybir.dt.bfloat16
    i32 = mybir.dt.int32
    Alu = mybir.AluOpType
    Act = mybir.ActivationFunctionType

    nc = bacc.Bacc("TRN2", target_bir_lowering=False, num_devices=NCORES)

    xt = nc.dram_tensor("xt", [D, H], f32, kind="ExternalInput")
    wt = nc.dram_tensor("wt", [D, E], f32, kind="ExternalInput")
    mo = nc.dram_tensor("mo", [2, NF], f32, kind="ExternalOutput")
    po = nc.dram_tensor("po", [2, NF], f32, kind="ExternalOutput")
    co = nc.dram_tensor("co", [E, 1], f32, kind="ExternalOutput")
    DEBUG = bool(int(os.environ.get("KDEBUG", "0")))
    if DEBUG:
        ro = nc.dram_tensor("ro", [128, NF], f32, kind="ExternalOutput")
        t0o = nc.dram_tensor("t0o", [E, 1], f32, kind="ExternalOutput")

    with TileContext(nc) as tc:
        with (
            tc.tile_pool(name="persist", bufs=1) as pp,
            tc.tile_pool(name="work", bufs=1) as wp,
            tc.tile_pool(name="stream", bufs=4) as sp,
            tc.tile_pool(name="small", bufs=2) as smp,
            tc.tile_pool(name="dram", bufs=1, space="DRAM") as dp,
        ):
            # ---------------- constants (issued early; overlap phase A DMA) ----
            pi = pp.tile([128, 1], i32, tag="pi")
            nc.gpsimd.iota(pi[:], pattern=[[0, 1]], base=0, channel_multiplier=1)
            pif = pp.tile([128, 1], f32, tag="pif")
            nc.vector.tensor_copy(pif[:], pi[:])
            hp = pp.tile([128, 1], f32, tag="hp")
            nc.vector.tensor_scalar(hp[:], pif[:], 64.0, None, op0=Alu.is_ge)
            jp = pp.tile([128, 1], f32, tag="jp")
            nc.vector.scalar_tensor_tensor(
                jp[:], hp[:], -64.0, pif[:], op0=Alu.mult, op1=Alu.add
            )
            ci = pp.tile([128, 128], i32, tag="ci")
            nc.gpsimd.iota(ci[:], pattern=[[1, 128]], base=0, channel_multiplier=0)
            cif = pp.tile([128, 128], f32, tag="cif")
            nc.vector.tensor_copy(cif[:], ci[:])
            hc = pp.tile([128, 128], f32, tag="hc")
            nc.vector.tensor_scalar(hc[:], cif[:], 64.0, None, op0=Alu.is_ge)
            jcol = pp.tile([128, 128], f32, tag="jcol")
            nc.vector.scalar_tensor_tensor(
                jcol[:], hc[:], -64.0, cif[:], op0=Alu.mult, op1=Alu.add
            )
            cond1 = wp.tile([128, 128], f32, tag="cond1")
            nc.vector.tensor_scalar(cond1[:], jcol[:], jp[:], None, op0=Alu.is_lt)
            cond2 = wp.tile([128, 128], f32, tag="cond2")
            nc.vector.tensor_scalar(cond2[:], hc[:], hp[:], None, op0=Alu.is_equal)
            # ustrict[p=j'+64h', c=j+64h] = -BIG if j' > j and h' == h
            ustrict = pp.tile([128, 128], bf16, tag="ustrict")
            nc.vector.scalar_tensor_tensor(
                ustrict[:], cond1[:], -BIGSEL, cond2[:], op0=Alu.mult, op1=Alu.mult
            )
            ident128 = pp.tile([128, 128], f32, tag="ident128")
            make_identity(nc, ident128)
            # Wz [128, 2]: col0 = 1-hp, col1 = hp (u-half fold for column sums)
            Wz = pp.tile([128, 2], f32, tag="Wz")
            nc.vector.tensor_scalar(
                Wz[:, 0:1], hp[:], -1.0, 1.0, op0=Alu.mult, op1=Alu.add
            )
            nc.vector.tensor_copy(Wz[:, 1:2], hp[:])
            # Wb [2, 128]: row p -> 1 where column's u-half == p (broadcast up)
            ci2 = pp.tile([2, 128], i32, tag="ci2")
            nc.gpsimd.iota(ci2[:], pattern=[[1, 128]], base=0, channel_multiplier=0)
            ci2f = pp.tile([2, 128], f32, tag="ci2f")
            nc.vector.tensor_copy(ci2f[:], ci2[:])
            hc2 = pp.tile([2, 128], f32, tag="hc2")
            nc.vector.tensor_scalar(hc2[:], ci2f[:], 64.0, None, op0=Alu.is_ge)
            pi2 = pp.tile([2, 1], i32, tag="pi2")
            nc.gpsimd.iota(pi2[:], pattern=[[0, 1]], base=0, channel_multiplier=1)
            pi2f = pp.tile([2, 1], f32, tag="pi2f")
            nc.vector.tensor_copy(pi2f[:], pi2[:])
            Wb = pp.tile([2, 128], f32, tag="Wb")
            nc.vector.tensor_scalar(Wb[:], hc2[:], pi2f[:], None, op0=Alu.is_equal)
            # Wm [128, 2] bf16: col u = jp * (hp == u) (priority readout)
            Wm = pp.tile([128, 2], bf16, tag="Wm")
            nc.vector.tensor_tensor(Wm[:, 0:1], jp[:], Wz[:, 0:1], op=Alu.mult)
            nc.vector.tensor_tensor(Wm[:, 1:2], jp[:], hp[:], op=Alu.mult)
            # iota17 [64, 17] for cand indexing
            i17 = pp.tile([64, 17], i32, tag="i17")
            nc.gpsimd.iota(i17[:], pattern=[[1, 17]], base=0, channel_multiplier=0)
            iota17 = pp.tile([64, 17], f32, tag="iota17")
            nc.vector.tensor_copy(iota17[:], i17[:])
            # union-kth scratch (pads preset here, filled after AllGather-2)
            apad = wp.tile([64, 65], f32, tag="apad")
            brev = wp.tile([64, 65], f32, tag="brev")
            nc.vector.memset(apad[:, 0:1], 1e38)
            nc.vector.memset(brev[:, 64:], 1e38)

            # ---------------- Phase A: logits -> log-softmax ----------------
            # wtbig[p, dc, 0:64] = wt[dc*128+p, :]; [64:128] = 0; [128:192] = wt
            # WtLow(dc) = wtbig[:, dc, 0:128]  -> expert channels 0-63 (u=0)
            # WtHigh(dc) = wtbig[:, dc, 64:192] -> channels 64-127 (u=1)
            r2 = pp.tile([128, NF], f32, tag="r2")   # full folded s
            with tc.tile_pool(name="phA", bufs=1) as pa:
                # wt_sb[p, dc*64+e] = wt[dc*128+p, e]; u=1 logits land on psum
                # partitions 64:128 via output-AP partition offset (64-out fp32
                # matmuls run 2x faster than 128-out: 429ns vs 852ns per 512)
                wt_sb = pa.tile([128, 32 * E], f32, tag="wt")
                nc.sync.dma_start(
                    wt_sb[:].rearrange("p (c e) -> p c e", e=E),
                    wt[:].rearrange("(c p) e -> p c e", p=128),
                )
                with tc.tile_pool(name="plog", bufs=1, space="PSUM") as plog_pool:
                    psumA = plog_pool.tile([128, 1024], f32, tag="plog")
                    for dd in range(16):
                        # 2MB double-chunk per DMA; alternate trigger engines so
                        # descriptor issue does not serialize on one sequencer
                        xchunk = sp.tile([128, 2 * H], f32, tag="xchunk")
                        eng = nc.sync if dd % 2 == 0 else nc.scalar
                        eng.dma_start(
                            xchunk[:].rearrange("p (s t) -> p s t", t=H),
                            xt[dd * 256: (dd + 1) * 256, :].rearrange(
                                "(s p) t -> p s t", p=128
                            ),
                        )
                        for sub in range(2):
                            dc = dd * 2 + sub
                            wsl = slice(dc * E, (dc + 1) * E)
                            xbase = sub * H
                            for cg in range(2):
                                sl = slice(xbase + cg * 512, xbase + (cg + 1) * 512)
                                nc.tensor.matmul(
                                    psumA[0:64, cg * 512: (cg + 1) * 512],
                                    wt_sb[:, wsl],
                                    xchunk[:, sl],
                                    start=(dc == 0), stop=(dc == 31),
                                )
                                nc.tensor.matmul(
                                    psumA[64:128, cg * 512: (cg + 1) * 512],
                                    wt_sb[:, wsl],
                                    xchunk[:, xbase + 1024 + cg * 512: xbase + 1024 + (cg + 1) * 512],
                                    start=(dc == 0), stop=(dc == 31),
                                )
                    # Z per (u, col) by folding expert partitions; then ln
                    expT = wp.tile([128, 1024], f32, tag="expT")
                    nc.scalar.activation(expT[:], psumA[:], Act.Exp)
                    lnZ = wp.tile([2, 1024], f32, tag="lnZ")
                    with tc.tile_pool(name="pz", bufs=1, space="PSUM") as pz_pool:
                        pz = pz_pool.tile([2, 1024], f32, tag="pz")
                        for cg in range(2):
                            sl = slice(cg * 512, (cg + 1) * 512)
                            nc.tensor.matmul(
                                pz[:, sl], Wz[:], expT[:, sl], start=True, stop=True
                            )
                        nc.scalar.activation(lnZ[:], pz[:], Act.Ln)
                    with tc.tile_pool(name="pb", bufs=1, space="PSUM") as pb_pool:
                        lnZb = pb_pool.tile([128, 1024], f32, tag="lnZb")
                        for cg in range(2):
                            sl = slice(cg * 512, (cg + 1) * 512)
                            nc.tensor.matmul(
                                lnZb[:, sl], Wb[:], lnZ[:, sl], start=True, stop=True
                            )
                        # DVE can read only one PSUM input; stage lnZb via the
                        # (otherwise idle) scalar engine
                        lnZbs = wp.tile([128, 1024], f32, tag="lnZbs")
                        nc.scalar.activation(lnZbs[:], lnZb[:], Act.Copy)
                        # s (own fold) = logits - lnZ -> r2 cols 0:1024
                        nc.vector.tensor_tensor(
                            r2[:, 0:1024], psumA[:], lnZbs[:], op=Alu.subtract
                        )

            # ---------------- AllGather 1: s-tile (overlaps with init) -------
            agin1 = dp.tile([128, 1024], f32)
            agout1 = dp.tile([2, 128, 1024], f32)
            nc.scalar.dma_start(agin1[:], r2[:, 0:1024])
            nc.gpsimd.collective_compute(
                "AllGather",
                mybir.AluOpType.bypass,
                replica_groups=[[0, 4], [1, 5], [2, 6], [3, 7]],
                ins=[agin1.opt()],
                outs=[agout1.opt()],
            )

            # ---------------- Init: per-(j,u) top-64 of own half -------------
            cand2 = wp.tile([128, 64], f32, tag="cand2")
            wrkA = wp.tile([128, 1024], f32, tag="wrkA")
            wrkB = wp.tile([128, 1024], f32, tag="wrkB")
            nc.vector.tensor_copy(wrkA[:], r2[:, 0:1024])
            cur, nxt = wrkA, wrkB
            for rnd in range(8):
                m8 = smp.tile([128, 8], f32, tag="m8")
                nc.vector.max(m8[:], cur[:])
                nc.vector.tensor_copy(cand2[:, rnd * 8: rnd * 8 + 8], m8[:])
                if rnd < 7:
                    nc.vector.match_replace(
                        out=nxt[:], in_to_replace=m8[:], in_values=cur[:],
                        imm_value=-1e38,
                    )
                    cur, nxt = nxt, cur

            # gathered s-tiles -> r2 full (replica order; own rewrite harmless)
            for g in range(2):
                nc.sync.dma_start(
                    r2[:, g * 1024: (g + 1) * 1024], agout1[g, :, :]
                )

            # fold own 2 sorted-64 lists -> [64, 128] via partition-offset
            # copies; merge-sort to a single sorted top-64 (runs under
            # AllGather-1's link transfer)
            poolsb = wp.tile([64, 128], f32, tag="poolsb")
            nc.vector.tensor_copy(poolsb[:, 0:64], cand2[0:64, :])
            nc.vector.tensor_copy(poolsb[:, 64:128], cand2[64:128, :])
            poolwk = wp.tile([64, 128], f32, tag="poolwk")
            candOwn = wp.tile([64, 64], f32, tag="candOwn")
            curp, nxtp = poolsb, poolwk
            for rnd in range(8):
                m8b = smp.tile([64, 8], f32, tag="m8b")
                nc.vector.max(m8b[:], curp[:])
                nc.vector.tensor_copy(candOwn[:, rnd * 8: rnd * 8 + 8], m8b[:])
                if rnd < 7:
                    nc.vector.match_replace(
                        out=nxtp[:], in_to_replace=m8b[:], in_values=curp[:],
                        imm_value=-1e38,
                    )
                    curp, nxtp = nxtp, curp

            # ---------------- AllGather 2: merged own top-64 (16KB) ----------
            agin2 = dp.tile([64, 64], f32)
            agout2 = dp.tile([2, 64, 64], f32)
            nc.scalar.dma_start(agin2[:], candOwn[:])
            nc.gpsimd.collective_compute(
                "AllGather",
                mybir.AluOpType.bypass,
                replica_groups=[[0, 4], [1, 5], [2, 6], [3, 7]],
                ins=[agin2.opt()],
                outs=[agout2.opt()],
            )
            candAB = wp.tile([64, 128], f32, tag="candAB")
            for g in range(2):
                nc.scalar.dma_start(
                    candAB[:, g * 64: (g + 1) * 64], agout2[g, :, :]
                )

            # t0 = 64th of union of two sorted-64 lists:
            # max_{i+j=64} min(A_i, B_j) with A_0 = B_0 = +inf
            nc.vector.tensor_copy(apad[:, 1:], candAB[:, 0:64])
            nc.vector.tensor_copy(brev[:, 0:64], candAB[:, 64:128][:, ::-1])
            tmin = wp.tile([64, 65], f32, tag="tmin")
            nc.vector.tensor_tensor(tmin[:], apad[:], brev[:], op=Alu.min)
            t_vec = pp.tile([64, 1], f32, tag="t")
            nc.vector.tensor_reduce(
                t_vec[:], tmin[:], axis=mybir.AxisListType.X, op=Alu.max
            )

            if DEBUG:
                nc.sync.dma_start(ro[:], r2[:])
                nc.sync.dma_start(t0o[:], t_vec[:])

            # t broadcast to both u-halves (partition-offset copies)
            t2sb = pp.tile([128, 1], f32, tag="t2sbi", name="t2_init")
            nc.vector.tensor_copy(t2sb[0:64, :], t_vec[:])
            nc.vector.tensor_copy(t2sb[64:128, :], t_vec[:])

            # wave 0: raw claims mask only (steal matmul would be on zeros)
            msk = pp.tile([128, NF], bf16, tag="mskA", name="msk_init")
            nc.vector.tensor_scalar(msk[:], r2[:], t2sb[:], None, op0=Alu.is_ge)

            candp = smp.tile([128, 16], f32, tag="candp")
            NCH = 2
            CW = NF // NCH  # 1024

            # ---------------- waves ----------------
            # pm PSUM pool is global (bufs=3 = 6 banks): chunk-0's identity
            # matmul for the NEXT wave is prefetched while the current wave's
            # DVE work runs, so after mskn lands only the (bf16, fast) steal
            # matmul remains on the critical path.
            wave_pools = tc.tile_pool(name="pmglob", bufs=2, space="PSUM")
            cw_pools = tc.tile_pool(name="pcw", bufs=1, space="PSUM")
            pmp = wave_pools.__enter__()
            pcw = cw_pools.__enter__()
            pm_pre = pmp.tile([128, CW], f32, tag="pm", name="pm_w1_c0")
            for cg in range(CW // 512):
                psl = slice(cg * 512, (cg + 1) * 512)
                nc.tensor.matmul(
                    pm_pre[:, psl], ident128[:], r2[:, psl],
                    start=True, stop=False,
                )
            for wv in range(1, WAVES + 2):
                last = wv == WAVES + 1
                cntp2 = smp.tile([128, NCH], f32, tag="cntp2")
                if last:
                    msk01 = pp.tile([128, NF], bf16, tag="msk01")
                for ch in range(NCH):
                    if ch == 0:
                        pm = pm_pre
                    else:
                        pm = pmp.tile([128, CW], f32, tag="pm",
                                      name=f"pm_w{wv}_c{ch}")
                    for cg in range(CW // 512):
                        sl = slice(ch * CW + cg * 512, ch * CW + (cg + 1) * 512)
                        psl = slice(cg * 512, (cg + 1) * 512)
                        if ch != 0:
                            nc.tensor.matmul(
                                pm[:, psl], ident128[:], r2[:, sl],
                                start=True, stop=False,
                            )
                        nc.tensor.matmul(
                            pm[:, psl], ustrict[:], msk[:, sl],
                            start=False, stop=True,
                        )
                    sl = slice(ch * CW, (ch + 1) * CW)
                    if last:
                        nc.vector.tensor_scalar(
                            msk01[:, sl], pm[:], t2sb[:], None,
                            op0=Alu.is_ge, op1=Alu.add,
                            accum_out=cntp2[:, ch: ch + 1],
                        )
                    elif True:
                        selm = smp.tile([128, CW], bf16, tag="selm")
                        nc.vector.tensor_scalar(
                            selm[:], pm[:], t2sb[:], None,
                            op0=Alu.is_ge, op1=Alu.add,
                            accum_out=cntp2[:, ch: ch + 1],
                        )
                        wchunk = smp.tile([128, CW], f32, tag="wchunk")
                        nc.vector.scalar_tensor_tensor(
                            wchunk[:], selm[:], -BIGSEL, pm[:],
                            op0=Alu.mult, op1=Alu.add,
                        )
                        nc.vector.max(candp[:, ch * 8: ch * 8 + 8], wchunk[:])

                # prefetch next wave chunk-0 identity sums (no data deps; runs
                # on PE while DVE processes this wave)
                if not last:
                    pm_pre = pmp.tile([128, CW], f32, tag="pm",
                                      name=f"pm_w{wv + 1}_c0")
                    for cg in range(CW // 512):
                        psl = slice(cg * 512, (cg + 1) * 512)
                        nc.tensor.matmul(
                            pm_pre[:, psl], ident128[:], r2[:, psl],
                            start=True, stop=False,
                        )

                # counts: reduce chunks, then fold the two u-halves with
                # partition-offset copy + same-base add (no PE round trip)
                cnt2 = smp.tile([128, 1], f32, tag="cnt2")
                nc.vector.tensor_reduce(
                    cnt2[:], cntp2[:], axis=mybir.AxisListType.X, op=Alu.add
                )
                if True:
                    cnthi = smp.tile([64, 1], f32, tag="cnthi")
                    nc.vector.tensor_copy(cnthi[:], cnt2[64:128, :])
                    cntf = smp.tile([64, 1], f32, tag="cntfsb")
                    nc.vector.tensor_tensor(
                        cntf[:], cnt2[0:64, :], cnthi[:], op=Alu.add
                    )
                    if last:
                        nc.sync.dma_start(co[:], cntf[:])
                        msk = msk01
                        break

                    # candidate pool [64, 32] -> top-16 into cand17[1:17]
                    cand17 = smp.tile([64, 17], f32, tag="cand17")
                    nc.vector.tensor_copy(cand17[:, 0:1], t_vec[:])
                    pool32 = smp.tile([64, 32], f32, tag="pool32")
                    nc.vector.tensor_copy(pool32[:, 0:16], candp[0:64, :])
                    nc.vector.tensor_copy(pool32[:, 16:32], candp[64:128, :])
                    nc.vector.max(cand17[:, 1:9], pool32[:])
                    pool32b = smp.tile([64, 32], f32, tag="pool32b")
                    nc.vector.match_replace(
                        out=pool32b[:], in_to_replace=cand17[:, 1:9],
                        in_values=pool32[:], imm_value=-1e38,
                    )
                    nc.vector.max(cand17[:, 9:17], pool32b[:])

                    # d = clamp(64 - cnt, 0, DMAX); t = cand17[d]
                    dmv = smp.tile([64, 1], f32, tag="dmv")
                    nc.vector.tensor_scalar(
                        dmv[:], cntf[:], -1.0, 64.0,
                        op0=Alu.mult, op1=Alu.add,
                    )
                    nc.vector.tensor_scalar_min(dmv[:], dmv[:], float(DMAX))
                    nc.vector.tensor_scalar_max(dmv[:], dmv[:], 0.0)
                    oh = smp.tile([64, 17], f32, tag="oh")
                    nc.vector.tensor_scalar(
                        oh[:], iota17[:], dmv[:], None, op0=Alu.is_equal
                    )
                    tsel = smp.tile([64, 17], f32, tag="tsel")
                    nc.vector.tensor_tensor(tsel[:], oh[:], cand17[:], op=Alu.mult)
                    t_vec = pp.tile([64, 1], f32, tag=f"t{wv % 2}", name=f"tvec{wv}")
                    nc.vector.tensor_reduce(
                        t_vec[:], tsel[:], axis=mybir.AxisListType.X, op=Alu.add
                    )
                    t2sb = pp.tile([128, 1], f32, tag=f"t2sb{wv % 2}",
                                   name=f"t2_{wv}")
                    nc.vector.tensor_copy(t2sb[0:64, :], t_vec[:])
                    nc.vector.tensor_copy(t2sb[64:128, :], t_vec[:])
                mskn = pp.tile([128, NF], bf16, tag=f"msk{wv % 2}", name=f"mskn{wv}")
                for ch in range(NCH):
                    sl = slice(ch * CW, (ch + 1) * CW)
                    nc.vector.tensor_scalar(
                        mskn[:, sl], r2[:, sl], t2sb[:], None, op0=Alu.is_ge
                    )
                msk = mskn

            cw_pools.__exit__(None, None, None)
            wave_pools.__exit__(None, None, None)

            # ---------------- outputs ----------------
            psel = wp.tile([128, NF], f32, tag="psel")
            nc.vector.tensor_tensor(psel[:], r2[:], msk[:], op=Alu.mult)
            mo_sb = wp.tile([2, NF], f32, tag="mo")
            po_sb = wp.tile([2, NF], f32, tag="po")
            with tc.tile_pool(name="pout", bufs=4, space="PSUM") as pop:
                for ch in range(NF // 512):
                    sl = slice(ch * 512, (ch + 1) * 512)
                    pmm = pop.tile([2, 512], f32, tag="pmm")
                    nc.tensor.matmul(
                        pmm[:], Wm[:], msk[:, sl], start=True, stop=True
                    )
                    nc.vector.tensor_copy(mo_sb[:, sl], pmm[:])
                    ppp = pop.tile([2, 512], f32, tag="ppp")
                    nc.tensor.matmul(
                        ppp[:], Wz[:], psel[:, sl], start=True, stop=True
                    )
                    nc.scalar.activation(po_sb[:, sl], ppp[:], Act.Exp)
            nc.sync.dma_start(mo[:], mo_sb[:])
            nc.sync.dma_start(po[:], po_sb[:])

    nc.compile()
    return nc


def kernel(x, W, c):
    global LAST_EXEC_NS
    from concourse import bass_utils

    x = np.asarray(x, dtype=np.float32)
    W = np.asarray(W, dtype=np.float32)

    if "nc" not in _cache:
        _cache["nc"] = _build_program()
    nc = _cache["nc"]

    wt_host = np.ascontiguousarray(W.T)  # [D, E]
    in_maps = []
    for core in range(NCORES):
        b, h = core % B, core // B
        xt_host = np.ascontiguousarray(x[b, h * H: (h + 1) * H, :].T)  # [D, H]
        in_maps.append({"xt": xt_host, "wt": wt_host})

    trace = TRACE
    if trace:
        _install_ntff_hook()
    res = bass_utils.run_bass_kernel_spmd(
        nc, in_maps, core_ids=list(range(NCORES)), trace=trace
    )
    LAST_EXEC_NS = res.exec_time_ns

    M = np.zeros((B, N), dtype=np.int32)
    P = np.zeros((B, N), dtype=np.float32)
    for b in range(B):
        out = res.results[b]
        cnt = out["co"][:, 0]
        if not np.allclose(cnt, 64.0):
            print(f"[kernel] WARNING: batch {b} expert counts != 64: "
                  f"min={cnt.min()} max={cnt.max()}", file=sys.stderr)
        # core b has h=0: cols 0:1024 = tokens 0:2048 folded (u = lt//1024,
        # col = lt%1024); cols 1024:2048 = tokens 2048:4096 folded.
        m2 = out["mo"]  # [2, 2048]
        p2 = out["po"]
        M[b, :] = np.rint(m2[_u_of_n, _col_of_n]).astype(np.int32)
        P[b, :] = p2[_u_of_n, _col_of_n].astype(np.float32)
    return M, P
